# revision 37
# baseline (speedup 1.0000x reference)
"""Multi-Query Attention (b=1, seq=2048, dim=2048, 16 heads, head_dim=128)
on 8 Trainium2 NeuronCores.

Sharding: sequence-parallel over queries. Core c computes output rows
[256c, 256c+256). The K/V projections (small) are replicated on every
core; q-proj / attention / out-proj are disjoint per core, so there are
no collectives at all. The host pre-transposes + pre-casts weights and
activations to bf16; per-core outputs are row-blocks concatenated on the
host.

On-chip layout is "transposed world": activations are kept as
[feature, seq] so every matmul contracts over the partition dim.

  kT   [128hd, 2048k]  = sum_i WkT_i^T @ xT_i           (scaled 1/sqrt(hd))
  vT   [128hd, 2048k]  similarly, then PE-transposed to v [2048k, 128hd]
  q    [256q, 2048f]   = sum_i xTq_i^T @ WqT_i, PE-transposed per head to
  qT_h [128hd, 256q]
  sT_h [128k, 256q]    = kT_blk^T @ qT_h   per key block  (16 per head)
  eT_h = exp(sT)       ACT, PSUM->SBUF bf16 (no max-subtract: logits ~ N(0,1))
  denom[1, 256q]       = ones_col^T @ eT (PE partition-reduction, accum)
  pv   [128hd, 256q]   = sum_k v_blk^T @ eT_blk
  attnT_h = pv * bcast(1/denom)   (recip on DVE, PE ones-row broadcast)
  final[256q, 2048]    = sum_i attnT_i^T @ WoT_i
"""

import os
import sys

import numpy as np
import ml_dtypes

for _p in ("/opt/trn_rl_repo",):
    if _p not in sys.path and os.path.isdir(_p):
        sys.path.insert(0, _p)

DIM = 2048
SEQ = 2048
HEADS = 16
HD = 128
P = 128
NCORES = 8
QB = SEQ // NCORES  # 256 queries per core
NT = DIM // P  # 16 feature tiles
NKT = SEQ // P  # 16 key tiles
SCALE = 1.0 / float(np.sqrt(HD))

_BUILD_CACHE = {}


def _build():
    if "nc" in _BUILD_CACHE:
        return _BUILD_CACHE["nc"]

    import concourse.bass as bass
    import concourse.tile as tile
    from concourse import bacc, mybir
    from concourse.masks import make_identity

    bf16 = mybir.dt.bfloat16
    f32 = mybir.dt.float32
    Exp = mybir.ActivationFunctionType.Exp
    Copy = mybir.ActivationFunctionType.Copy

    nc = bacc.Bacc(None, target_bir_lowering=False, debug=False)

    xT_d = nc.dram_tensor("xT", [DIM, SEQ], bf16, kind="ExternalInput").ap()
    # pre-packed on host: [p, i, :] = strip i, partition p (contiguous DMA)
    xTq_d = nc.dram_tensor("xTq", [P, NT, QB], bf16, kind="ExternalInput").ap()
    wkT_d = nc.dram_tensor("wkT", [P, NT, HD], bf16, kind="ExternalInput").ap()
    wvT_d = nc.dram_tensor("wvT", [P, NT, HD], bf16, kind="ExternalInput").ap()
    wqT_d = nc.dram_tensor("wqT", [DIM, DIM], bf16, kind="ExternalInput").ap()
    woT_d = nc.dram_tensor("woT", [DIM, DIM], bf16, kind="ExternalInput").ap()
    out_d = nc.dram_tensor("out", [QB, DIM], f32, kind="ExternalOutput").ap()

    from contextlib import ExitStack

    with tile.TileContext(nc) as tc, ExitStack() as ctx:
        const = ctx.enter_context(tc.tile_pool(name="const", bufs=1))
        xres = ctx.enter_context(tc.tile_pool(name="xres", bufs=1))
        acts = ctx.enter_context(tc.tile_pool(name="acts", bufs=1))
        wstream = ctx.enter_context(tc.tile_pool(name="wstream", bufs=6))
        expp = ctx.enter_context(tc.tile_pool(name="expp", bufs=4))
        small = ctx.enter_context(tc.tile_pool(name="small", bufs=4))
        outp = ctx.enter_context(tc.tile_pool(name="outp", bufs=3))

        # ---- constants ----
        ident = const.tile([P, P], bf16)
        make_identity(nc, ident)
        ones_col = const.tile([P, 1], f32)
        nc.vector.memset(ones_col, 1.0)

        # ---- resident loads ----
        # small tensors first so the first matmuls aren't queued behind
        # the 8MB of xT strips
        wk_sb = xres.tile([P, NT, HD], bf16)
        nc.gpsimd.dma_start(out=wk_sb, in_=wkT_d)
        wv_sb = xres.tile([P, NT, HD], bf16)
        nc.gpsimd.dma_start(out=wv_sb, in_=wvT_d)
        xq_sb = xres.tile([P, NT, QB], bf16)
        nc.gpsimd.dma_start(out=xq_sb, in_=xTq_d)

        kT_sb = acts.tile([P, SEQ], bf16)  # [hd, keys], pre-scaled
        vT_sb = acts.tile([P, SEQ], bf16)  # [hd, keys]
        v_sb = acts.tile([P, NKT, HD], bf16)  # v_sb[:, kt, :] = v block kt
        q_sb = acts.tile([P, 2, DIM], bf16)  # q rows [256, 2048] as 2 tiles
        qT_sb = acts.tile([P, HEADS, QB], bf16)  # qT_sb[:, h, :] = [hd, 256]
        attn_sb = acts.tile([P, HEADS, QB], bf16)  # attnT per head

        # ---- phase B: kT / vT projections (full seq, replicated) ----
        # xT lives in its own pool, released after this phase so Wo can
        # reuse the space.
        with tc.tile_pool(name="xtp", bufs=1) as xtp:
            x_sb = xtp.tile([P, NT, SEQ], bf16)  # xT strips: x_sb[p, i, s]
            for i in range(NT):
                eng = nc.gpsimd if i % 4 == 3 else nc.sync
                eng.dma_start(out=x_sb[:, i, :], in_=xT_d[i * P : (i + 1) * P, :])

            # all 8 accumulation chains advance together so each strip is
            # consumed the moment it arrives (B finishes with the DMA)
            with tc.tile_pool(name="pbkv", bufs=1, space="PSUM") as pbkv:
                pk = [
                    pbkv.tile([P, 512], f32, name=f"pk{n}", tag=f"pk{n}")
                    for n in range(4)
                ]
                pv = [
                    pbkv.tile([P, 512], f32, name=f"pvb{n}", tag=f"pvb{n}")
                    for n in range(4)
                ]
                for i in range(NT):
                    for n in range(SEQ // 512):
                        nc.tensor.matmul(
                            pk[n],
                            lhsT=wk_sb[:, i, :],
                            rhs=x_sb[:, i, n * 512 : (n + 1) * 512],
                            start=(i == 0),
                            stop=(i == NT - 1),
                        )
                        nc.tensor.matmul(
                            pv[n],
                            lhsT=wv_sb[:, i, :],
                            rhs=x_sb[:, i, n * 512 : (n + 1) * 512],
                            start=(i == 0),
                            stop=(i == NT - 1),
                        )
                for n in range(SEQ // 512):
                    nc.scalar.activation(
                        kT_sb[:, n * 512 : (n + 1) * 512], pk[n], Copy, scale=SCALE
                    )
                    nc.scalar.activation(vT_sb[:, n * 512 : (n + 1) * 512], pv[n], Copy)

        # v = vT^T, block-transposed on the PE
        with tc.tile_pool(name="ptrv", bufs=2, space="PSUM") as ptrv:
            for kt in range(NKT):
                pt = ptrv.tile([P, P], bf16)
                nc.tensor.transpose(pt, vT_sb[:, kt * P : (kt + 1) * P], ident)
                nc.vector.tensor_copy(v_sb[:, kt, :], pt)

        # Wo resident, reusing the SBUF space released by xT; DMAs issued
        # on the scalar queue so they don't contend with wq streaming
        wores = ctx.enter_context(tc.tile_pool(name="wores", bufs=1))
        wo_sb = wores.tile([P, NT, DIM], bf16)
        for i in range(NT):
            nc.scalar.dma_start(out=wo_sb[:, i, :], in_=woT_d[i * P : (i + 1) * P, :])

        # ---- phase C: q projection for this core's 256 queries ----
        with tc.tile_pool(name="pq", bufs=1, space="PSUM") as pqp:
            pq = [pqp.tile([P, 512], f32, name=f"pq{j}", tag=f"pq{j}") for j in range(8)]
            for i in range(NT):
                wq_t = wstream.tile([P, DIM], bf16, tag="w")
                nc.sync.dma_start(out=wq_t, in_=wqT_d[i * P : (i + 1) * P, :])
                for m2 in range(2):
                    for nn in range(4):
                        nc.tensor.matmul(
                            pq[m2 * 4 + nn],
                            lhsT=xq_sb[:, i, m2 * P : (m2 + 1) * P],
                            rhs=wq_t[:, nn * 512 : (nn + 1) * 512],
                            start=(i == 0),
                            stop=(i == NT - 1),
                        )
            for m2 in range(2):
                for nn in range(4):
                    nc.scalar.activation(
                        q_sb[:, m2, nn * 512 : (nn + 1) * 512], pq[m2 * 4 + nn], Copy
                    )

        # ---- phase D: attention, two heads at a time ----
        # MQA: kT / v stationary operands are shared across heads, so the
        # moving side can span two heads' queries -> N=512 streams (keeps
        # the PE HAM-warm and halves instruction count).
        HP = 512  # 2 heads x QB queries
        with (
            tc.tile_pool(name="ps", bufs=2, space="PSUM") as psp,
            tc.tile_pool(name="pd", bufs=1, space="PSUM") as pdp,
            tc.tile_pool(name="ppv", bufs=2, space="PSUM") as ppvp,
            tc.tile_pool(name="ptq", bufs=1, space="PSUM") as ptqp,
        ):
            def emit_qT(h, m2):
                pt = ptqp.tile([P, P], bf16, name="ptq_t", tag="ptq_t")
                nc.tensor.transpose(pt, q_sb[:, m2, h * P : (h + 1) * P], ident)
                nc.vector.tensor_copy(qT_sb[:, h, m2 * P : (m2 + 1) * P], pt)

            # qT for pair 0 up front; later pairs' transposes are threaded
            # into the previous pair's matmul stream
            for half in range(2):
                for m2 in range(2):
                    emit_qT(half, m2)

            for hp in range(HEADS // 2):
                q_pair = qT_sb[:, 2 * hp : 2 * hp + 2, :].rearrange("p h q -> p (h q)")
                pv_acc = ppvp.tile([P, HP], f32)
                # per-key partial sums of exp accumulate on the (idle) DVE;
                # the PE only does one M=1 matmul per pair for the final
                # cross-partition reduction
                e_acc = small.tile([P, HP], f32, tag="eacc")
                for kt2 in range(NKT // 2):
                    ps_t = psp.tile([P, 2, 512], f32)  # two key blocks, 2 banks
                    ex_t = expp.tile([P, 2, 512], bf16)
                    for half in range(2):
                        kt = 2 * kt2 + half
                        nc.tensor.matmul(
                            ps_t[:, half, :],
                            lhsT=kT_sb[:, kt * P : (kt + 1) * P],
                            rhs=q_pair,
                            start=True,
                            stop=True,
                        )
                    nc.scalar.activation(
                        ex_t.rearrange("p a b -> p (a b)"),
                        ps_t.rearrange("p a b -> p (a b)"),
                        Exp,
                    )
                    for half in range(2):
                        kt = 2 * kt2 + half
                        eslice = ex_t[:, half, :]
                        if kt == 0:
                            nc.vector.tensor_copy(e_acc, eslice)
                        else:
                            nc.vector.tensor_add(e_acc, e_acc, eslice)
                        nc.tensor.matmul(
                            pv_acc,
                            lhsT=v_sb[:, kt, :],
                            rhs=eslice,
                            start=(kt == 0),
                            stop=(kt == NKT - 1),
                        )
                    # thread next pair's transposes mid-stream, spaced out
                    # so each PSUM slot recycles before the next transpose
                    if hp + 1 < HEADS // 2 and kt2 % 2 == 1:
                        j = kt2 // 2
                        emit_qT(2 * (hp + 1) + j // 2, j % 2)
                d_acc = pdp.tile([1, HP], f32)
                nc.tensor.matmul(d_acc, lhsT=ones_col, rhs=e_acc, start=True, stop=True)
                # evict unnormalized PV immediately (frees the PSUM bank;
                # bf16 is scale-free so normalizing after eviction is safe)
                raw_sb = small.tile([P, HP], bf16, tag="rawsb")
                nc.scalar.activation(raw_sb, pv_acc, Copy)
                # denominator -> all 128 partitions on the (idle) gpsimd
                d_sb = small.tile([1, HP], f32, tag="dsb")
                nc.vector.tensor_copy(d_sb, d_acc)
                d_bc = small.tile([P, HP], f32, tag="dbc")
                nc.gpsimd.partition_broadcast(d_bc, d_sb)
                r_sb = small.tile([P, HP], f32, tag="rsb")
                nc.vector.reciprocal(r_sb, d_bc)
                nc.vector.tensor_mul(
                    attn_sb[:, 2 * hp : 2 * hp + 2, :].rearrange("p h q -> p (h q)"),
                    raw_sb,
                    r_sb,
                )

        # ---- phase E: out projection -> final rows [256, 2048] ----
        # two output-column groups so group 0's eviction + store overlap
        # group 1's matmuls (Wo is SBUF-resident)
        with tc.tile_pool(name="po", bufs=2, space="PSUM") as pop:
            for g in range(2):
                po = [
                    pop.tile([P, 512], f32, name=f"po{g}_{j}", tag=f"po{j}")
                    for j in range(4)
                ]
                for i in range(NT):
                    for m2 in range(2):
                        for nn in range(2):
                            col = g * 1024 + nn * 512
                            nc.tensor.matmul(
                                po[m2 * 2 + nn],
                                lhsT=attn_sb[:, i, m2 * P : (m2 + 1) * P],
                                rhs=wo_sb[:, i, col : col + 512],
                                start=(i == 0),
                                stop=(i == NT - 1),
                            )
                for m2 in range(2):
                    for nn in range(2):
                        col = g * 1024 + nn * 512
                        o_sb = outp.tile([P, 512], f32)
                        nc.scalar.activation(o_sb, po[m2 * 2 + nn], Copy)
                        nc.sync.dma_start(
                            out=out_d[m2 * P : (m2 + 1) * P, col : col + 512],
                            in_=o_sb,
                        )

    nc.compile()
    _BUILD_CACHE["nc"] = nc
    return nc


def prep_in_maps(x, Wq, Wk, Wv, Wo):
    """Host-side preprocessing: transpose, cast to bf16, slice per core."""
    bf16 = ml_dtypes.bfloat16

    def pack_strips(a):
        # [NT*P, F] -> [P, NT, F]: row i*P+p lands at [p, i, :]
        nt = a.shape[0] // P
        return np.ascontiguousarray(a.reshape(nt, P, -1).transpose(1, 0, 2))

    x2 = np.ascontiguousarray(np.asarray(x, dtype=np.float32)[0].T).astype(bf16)
    wq = np.ascontiguousarray(np.asarray(Wq, dtype=np.float32).T).astype(bf16)
    wk = pack_strips(np.asarray(Wk, dtype=np.float32).T.astype(bf16))
    wv = pack_strips(np.asarray(Wv, dtype=np.float32).T.astype(bf16))
    wo = np.ascontiguousarray(np.asarray(Wo, dtype=np.float32).T).astype(bf16)
    in_maps = []
    for c in range(NCORES):
        in_maps.append(
            {
                "xT": x2,
                "xTq": pack_strips(x2[:, c * QB : (c + 1) * QB]),
                "wkT": wk,
                "wvT": wv,
                "wqT": wq,
                "woT": wo,
            }
        )
    return in_maps


def run_on_hw(in_maps, trace=False):
    from concourse.bass_utils import run_bass_kernel_spmd

    nc = _build()
    return run_bass_kernel_spmd(nc, in_maps, list(range(NCORES)), trace=trace)


def kernel(x, Wq, Wk, Wv, Wo):
    in_maps = prep_in_maps(x, Wq, Wk, Wv, Wo)
    res = run_on_hw(in_maps)
    out = np.concatenate([res.results[c]["out"] for c in range(NCORES)], axis=0)
    return out.astype(np.float32)[None]


# revision 43
# speedup vs baseline: 1.1705x; 1.1705x over previous
"""Multi-Query Attention (b=1, seq=2048, dim=2048, 16 heads, head_dim=128)
on 8 Trainium2 NeuronCores.

Sharding: sequence-parallel over queries. Core c computes output rows
[256c, 256c+256). The K/V projections (small) are replicated on every
core; q-proj / attention / out-proj are disjoint per core, so there are
no collectives at all. The host pre-transposes + pre-casts weights and
activations to bf16; per-core outputs are row-blocks concatenated on the
host.

On-chip layout is "transposed world": activations are kept as
[feature, seq] so every matmul contracts over the partition dim.

  kT   [128hd, 2048k]  = sum_i WkT_i^T @ xT_i           (scaled 1/sqrt(hd))
  vT   [128hd, 2048k]  similarly, then PE-transposed to v [2048k, 128hd]
  q    [256q, 2048f]   = sum_i xTq_i^T @ WqT_i, PE-transposed per head to
  qT_h [128hd, 256q]
  sT_h [128k, 256q]    = kT_blk^T @ qT_h   per key block  (16 per head)
  eT_h = exp(sT)       ACT, PSUM->SBUF bf16 (no max-subtract: logits ~ N(0,1))
  denom[1, 256q]       = ones_col^T @ eT (PE partition-reduction, accum)
  pv   [128hd, 256q]   = sum_k v_blk^T @ eT_blk
  attnT_h = pv * bcast(1/denom)   (recip on DVE, PE ones-row broadcast)
  final[256q, 2048]    = sum_i attnT_i^T @ WoT_i
"""

import os
import sys

import numpy as np
import ml_dtypes

for _p in ("/opt/trn_rl_repo",):
    if _p not in sys.path and os.path.isdir(_p):
        sys.path.insert(0, _p)

DIM = 2048
SEQ = 2048
HEADS = 16
HD = 128
P = 128
NCORES = 8
QB = SEQ // NCORES  # 256 queries per core
NT = DIM // P  # 16 feature tiles
NKT = SEQ // P  # 16 key tiles
SCALE = 1.0 / float(np.sqrt(HD))

_BUILD_CACHE = {}


def _build():
    if "nc" in _BUILD_CACHE:
        return _BUILD_CACHE["nc"]

    import concourse.bass as bass
    import concourse.tile as tile
    from concourse import bacc, mybir
    from concourse.masks import make_identity

    bf16 = mybir.dt.bfloat16
    f32 = mybir.dt.float32
    Exp = mybir.ActivationFunctionType.Exp
    Copy = mybir.ActivationFunctionType.Copy

    nc = bacc.Bacc(None, target_bir_lowering=False, debug=False)

    xT_d = nc.dram_tensor("xT", [DIM, SEQ], bf16, kind="ExternalInput").ap()
    # pre-packed on host: [p, i, :] = strip i, partition p (contiguous DMA)
    xTq_d = nc.dram_tensor("xTq", [P, NT, QB], bf16, kind="ExternalInput").ap()
    wkT_d = nc.dram_tensor("wkT", [P, NT, HD], bf16, kind="ExternalInput").ap()
    wvT_d = nc.dram_tensor("wvT", [P, NT, HD], bf16, kind="ExternalInput").ap()
    wqT_d = nc.dram_tensor("wqT", [DIM, DIM], bf16, kind="ExternalInput").ap()
    woT_d = nc.dram_tensor("woT", [DIM, DIM], bf16, kind="ExternalInput").ap()
    out_d = nc.dram_tensor("out", [QB, DIM], f32, kind="ExternalOutput").ap()

    from contextlib import ExitStack

    with tile.TileContext(nc) as tc, ExitStack() as ctx:
        const = ctx.enter_context(tc.tile_pool(name="const", bufs=1))
        xres = ctx.enter_context(tc.tile_pool(name="xres", bufs=1))
        acts = ctx.enter_context(tc.tile_pool(name="acts", bufs=1))
        wstream = ctx.enter_context(tc.tile_pool(name="wstream", bufs=10))
        expp = ctx.enter_context(tc.tile_pool(name="expp", bufs=4))
        small = ctx.enter_context(tc.tile_pool(name="small", bufs=4))
        outp = ctx.enter_context(tc.tile_pool(name="outp", bufs=3))

        # ---- constants ----
        ident = const.tile([P, P], bf16)
        make_identity(nc, ident)
        ones_col = const.tile([P, 1], bf16)
        nc.vector.memset(ones_col, 1.0)

        # ---- resident loads ----
        # small tensors first so the first matmuls aren't queued behind
        # the 8MB of xT strips
        wk_sb = xres.tile([P, NT, HD], bf16)
        nc.gpsimd.dma_start(out=wk_sb, in_=wkT_d)
        wv_sb = xres.tile([P, NT, HD], bf16)
        nc.gpsimd.dma_start(out=wv_sb, in_=wvT_d)
        xq_sb = xres.tile([P, NT, QB], bf16)
        nc.gpsimd.dma_start(out=xq_sb, in_=xTq_d)

        kT_sb = acts.tile([P, SEQ], bf16)  # [hd, keys], pre-scaled
        vT_sb = acts.tile([P, SEQ], bf16)  # [hd, keys]
        v_sb = acts.tile([P, NKT, HD], bf16)  # v_sb[:, kt, :] = v block kt
        q_sb = acts.tile([P, 2, DIM], bf16)  # q rows [256, 2048] as 2 tiles
        qT_sb = acts.tile([P, HEADS, QB], bf16)  # qT_sb[:, h, :] = [hd, 256]
        attn_sb = acts.tile([P, HEADS, QB], bf16)  # attnT per head

        # ---- phase B: kT / vT projections (full seq, replicated) ----
        # xT lives in its own pool, released after this phase so Wo can
        # reuse the space.
        with tc.tile_pool(name="xtp", bufs=1) as xtp:
            x_sb = xtp.tile([P, NT, SEQ], bf16)  # xT strips: x_sb[p, i, s]
            for i in range(NT):
                eng = nc.gpsimd if i % 4 == 3 else nc.sync
                eng.dma_start(out=x_sb[:, i, :], in_=xT_d[i * P : (i + 1) * P, :])

            # all 8 accumulation chains advance together so each strip is
            # consumed the moment it arrives (B finishes with the DMA)
            with tc.tile_pool(name="pbkv", bufs=1, space="PSUM") as pbkv:
                pk = [
                    pbkv.tile([P, 512], f32, name=f"pk{n}", tag=f"pk{n}")
                    for n in range(4)
                ]
                pv = [
                    pbkv.tile([P, 512], f32, name=f"pvb{n}", tag=f"pvb{n}")
                    for n in range(4)
                ]
                for i in range(NT - 1):
                    for n in range(SEQ // 512):
                        nc.tensor.matmul(
                            pk[n],
                            lhsT=wk_sb[:, i, :],
                            rhs=x_sb[:, i, n * 512 : (n + 1) * 512],
                            start=(i == 0),
                            stop=False,
                        )
                        nc.tensor.matmul(
                            pv[n],
                            lhsT=wv_sb[:, i, :],
                            rhs=x_sb[:, i, n * 512 : (n + 1) * 512],
                            start=(i == 0),
                            stop=False,
                        )
                # finish + evict chain by chain so evictions overlap the
                # remaining chains' matmuls (no bunched PSUM handoff)
                for n in range(SEQ // 512):
                    nc.tensor.matmul(
                        pk[n],
                        lhsT=wk_sb[:, NT - 1, :],
                        rhs=x_sb[:, NT - 1, n * 512 : (n + 1) * 512],
                        start=False,
                        stop=True,
                    )
                    nc.scalar.activation(
                        kT_sb[:, n * 512 : (n + 1) * 512], pk[n], Copy, scale=SCALE
                    )
                    nc.tensor.matmul(
                        pv[n],
                        lhsT=wv_sb[:, NT - 1, :],
                        rhs=x_sb[:, NT - 1, n * 512 : (n + 1) * 512],
                        start=False,
                        stop=True,
                    )
                    nc.scalar.activation(vT_sb[:, n * 512 : (n + 1) * 512], pv[n], Copy)

        # v = vT^T, block-transposed on the PE
        with tc.tile_pool(name="ptrv", bufs=2, space="PSUM") as ptrv:
            for kt in range(NKT):
                pt = ptrv.tile([P, P], bf16)
                nc.tensor.transpose(pt, vT_sb[:, kt * P : (kt + 1) * P], ident)
                nc.vector.tensor_copy(v_sb[:, kt, :], pt)

        # Wo resident, reusing the SBUF space released by xT; DMAs issued
        # on the scalar queue so they don't contend with wq streaming
        wores = ctx.enter_context(tc.tile_pool(name="wores", bufs=1))
        wo_sb = wores.tile([P, NT, DIM], bf16)
        for i in range(NT):
            nc.scalar.dma_start(out=wo_sb[:, i, :], in_=woT_d[i * P : (i + 1) * P, :])

        # ---- phase C: q projection for this core's 256 queries ----
        with tc.tile_pool(name="pq", bufs=1, space="PSUM") as pqp:
            pq = [pqp.tile([P, 512], f32, name=f"pq{j}", tag=f"pq{j}") for j in range(8)]
            for i in range(NT):
                wq_t = wstream.tile([P, DIM], bf16, tag="w")
                nc.sync.dma_start(out=wq_t, in_=wqT_d[i * P : (i + 1) * P, :])
                for m2 in range(2):
                    for nn in range(4):
                        nc.tensor.matmul(
                            pq[m2 * 4 + nn],
                            lhsT=xq_sb[:, i, m2 * P : (m2 + 1) * P],
                            rhs=wq_t[:, nn * 512 : (nn + 1) * 512],
                            start=(i == 0),
                            stop=(i == NT - 1),
                        )
            for m2 in range(2):
                for nn in range(4):
                    nc.scalar.activation(
                        q_sb[:, m2, nn * 512 : (nn + 1) * 512], pq[m2 * 4 + nn], Copy
                    )

        # ---- phase D: attention, two heads at a time ----
        # MQA: kT / v stationary operands are shared across heads, so the
        # moving side can span two heads' queries -> N=512 streams (keeps
        # the PE HAM-warm and halves instruction count).
        HP = 512  # 2 heads x QB queries
        with (
            tc.tile_pool(name="ps", bufs=2, space="PSUM") as psp,
            tc.tile_pool(name="pd", bufs=1, space="PSUM") as pdp,
            tc.tile_pool(name="ppv", bufs=2, space="PSUM") as ppvp,
            tc.tile_pool(name="ptq", bufs=1, space="PSUM") as ptqp,
        ):
            def emit_qT(h, m2):
                pt = ptqp.tile([P, P], bf16, name="ptq_t", tag="ptq_t")
                nc.tensor.transpose(pt, q_sb[:, m2, h * P : (h + 1) * P], ident)
                nc.vector.tensor_copy(qT_sb[:, h, m2 * P : (m2 + 1) * P], pt)

            # qT for pair 0 up front; later pairs' transposes are threaded
            # into the previous pair's matmul stream
            for half in range(2):
                for m2 in range(2):
                    emit_qT(half, m2)

            for hp in range(HEADS // 2):
                q_pair = qT_sb[:, 2 * hp : 2 * hp + 2, :].rearrange("p h q -> p (h q)")
                pv_acc = ppvp.tile([P, HP], f32)
                d_acc = pdp.tile([1, HP], f32)
                for kt2 in range(NKT // 2):
                    ps_t = psp.tile([P, 2, 512], f32)  # two key blocks, 2 banks
                    ex_t = expp.tile([P, 2, 512], bf16)
                    for half in range(2):
                        kt = 2 * kt2 + half
                        nc.tensor.matmul(
                            ps_t[:, half, :],
                            lhsT=kT_sb[:, kt * P : (kt + 1) * P],
                            rhs=q_pair,
                            start=True,
                            stop=True,
                        )
                    nc.scalar.activation(
                        ex_t.rearrange("p a b -> p (a b)"),
                        ps_t.rearrange("p a b -> p (a b)"),
                        Exp,
                    )
                    for half in range(2):
                        kt = 2 * kt2 + half
                        eslice = ex_t[:, half, :]
                        nc.tensor.matmul(
                            d_acc,
                            lhsT=ones_col,
                            rhs=eslice,
                            start=(kt == 0),
                            stop=(kt == NKT - 1),
                        )
                        nc.tensor.matmul(
                            pv_acc,
                            lhsT=v_sb[:, kt, :],
                            rhs=eslice,
                            start=(kt == 0),
                            stop=(kt == NKT - 1),
                        )
                    # thread next pair's transposes mid-stream, spaced out
                    # so each PSUM slot recycles before the next transpose
                    if hp + 1 < HEADS // 2 and kt2 % 2 == 1:
                        j = kt2 // 2
                        emit_qT(2 * (hp + 1) + j // 2, j % 2)
                # evict unnormalized PV immediately (frees the PSUM bank;
                # bf16 is scale-free so normalizing after eviction is safe)
                raw_sb = small.tile([P, HP], bf16, tag="rawsb")
                nc.scalar.activation(raw_sb, pv_acc, Copy)
                # denominator -> all 128 partitions on the (idle) gpsimd
                d_sb = small.tile([1, HP], f32, tag="dsb")
                nc.vector.tensor_copy(d_sb, d_acc)
                d_bc = small.tile([P, HP], f32, tag="dbc")
                nc.gpsimd.partition_broadcast(d_bc, d_sb)
                r_sb = small.tile([P, HP], f32, tag="rsb")
                nc.vector.reciprocal(r_sb, d_bc)
                nc.vector.tensor_mul(
                    attn_sb[:, 2 * hp : 2 * hp + 2, :].rearrange("p h q -> p (h q)"),
                    raw_sb,
                    r_sb,
                )

        # ---- phase E: out projection -> final rows [256, 2048] ----
        # two output-column groups so group 0's eviction + store overlap
        # group 1's matmuls (Wo is SBUF-resident)
        with tc.tile_pool(name="po", bufs=2, space="PSUM") as pop:
            for g in range(2):
                po = [
                    pop.tile([P, 512], f32, name=f"po{g}_{j}", tag=f"po{j}")
                    for j in range(4)
                ]
                for i in range(NT):
                    for m2 in range(2):
                        for nn in range(2):
                            col = g * 1024 + nn * 512
                            nc.tensor.matmul(
                                po[m2 * 2 + nn],
                                lhsT=attn_sb[:, i, m2 * P : (m2 + 1) * P],
                                rhs=wo_sb[:, i, col : col + 512],
                                start=(i == 0),
                                stop=(i == NT - 1),
                            )
                for m2 in range(2):
                    for nn in range(2):
                        col = g * 1024 + nn * 512
                        o_sb = outp.tile([P, 512], f32)
                        nc.scalar.activation(o_sb, po[m2 * 2 + nn], Copy)
                        nc.sync.dma_start(
                            out=out_d[m2 * P : (m2 + 1) * P, col : col + 512],
                            in_=o_sb,
                        )

    nc.compile()
    _BUILD_CACHE["nc"] = nc
    return nc


def prep_in_maps(x, Wq, Wk, Wv, Wo):
    """Host-side preprocessing: transpose, cast to bf16, slice per core."""
    bf16 = ml_dtypes.bfloat16

    def pack_strips(a):
        # [NT*P, F] -> [P, NT, F]: row i*P+p lands at [p, i, :]
        nt = a.shape[0] // P
        return np.ascontiguousarray(a.reshape(nt, P, -1).transpose(1, 0, 2))

    x2 = np.ascontiguousarray(np.asarray(x, dtype=np.float32)[0].T).astype(bf16)
    wq = np.ascontiguousarray(np.asarray(Wq, dtype=np.float32).T).astype(bf16)
    wk = pack_strips(np.asarray(Wk, dtype=np.float32).T.astype(bf16))
    wv = pack_strips(np.asarray(Wv, dtype=np.float32).T.astype(bf16))
    wo = np.ascontiguousarray(np.asarray(Wo, dtype=np.float32).T).astype(bf16)
    in_maps = []
    for c in range(NCORES):
        in_maps.append(
            {
                "xT": x2,
                "xTq": pack_strips(x2[:, c * QB : (c + 1) * QB]),
                "wkT": wk,
                "wvT": wv,
                "wqT": wq,
                "woT": wo,
            }
        )
    return in_maps


def run_on_hw(in_maps, trace=False):
    from concourse.bass_utils import run_bass_kernel_spmd

    nc = _build()
    return run_bass_kernel_spmd(nc, in_maps, list(range(NCORES)), trace=trace)


def kernel(x, Wq, Wk, Wv, Wo):
    in_maps = prep_in_maps(x, Wq, Wk, Wv, Wo)
    res = run_on_hw(in_maps)
    out = np.concatenate([res.results[c]["out"] for c in range(NCORES)], axis=0)
    return out.astype(np.float32)[None]
